# revision 30
# baseline (speedup 1.0000x reference)
"""Trainium2 Bass kernel for nn_Decoder (show-attend-tell style LSTM decoder).

v2 strategy: data-parallel over batch (32 seqs/core x 8 cores), zero collectives.
Built per-input (specialized on caption_lengths):
 - batches sorted by length desc, assigned round-robin to cores so every core's
   local batch list is still sorted desc; a SHARED per-step active count NP_t
   (= ceil(#globally-active/8), padded to mult of 4) is baked into the code.
   Finished lanes compute garbage (bounded) which the host discards.
 - enc resident in SBUF as fp8-e3m4 (zero-sum dithered quantization) -> awe
   matmuls read SBUF, no per-step enc DMA (was 33MB/step).
 - W_f_beta resident fp8-e3m4 (x256 scale, undone in sigmoid scale), W_hh
   resident bf16 (x2), W_ih enc-part: half resident bf16 + half streamed.
 - att1 (P1 output) in DRAM bf16, streamed per step (active prefix only).
 - relu offloaded to ACT; per-step work sized to NP_t everywhere.
 - Phase 5 vocab projection deferred to the end (as before).
"""
import numpy as np
import ml_dtypes
from contextlib import ExitStack

import concourse.bass as bass
import concourse.tile as tile
from concourse import bacc, mybir
from concourse.bass_utils import run_bass_kernel_spmd

F32 = mybir.dt.float32
BF16 = mybir.dt.bfloat16
FP8E3 = mybir.dt.float8e3
FP8E4 = mybir.dt.float8e4
NP_BF16 = ml_dtypes.bfloat16
NP_FP8E3 = ml_dtypes.float8_e3m4
NP_FP8E4 = ml_dtypes.float8_e4m3
DR = mybir.MatmulPerfMode.DoubleRow

AX = mybir.AxisListType
OP = mybir.AluOpType
AF = mybir.ActivationFunctionType

B, P, ENC, EMB, DEC, ATT, V, L = 256, 196, 2048, 512, 512, 512, 10000, 21
T = L - 1
NCORES = 8
BL = B // NCORES          # 32 seqs per core
BP = BL * P               # 6272
EC = ENC // 128           # 16
AT = ATT // 128           # 4
DC = DEC // 128           # 4
GT = (4 * DEC) // 128     # 16
G = 4 * DEC               # 2048
VT = 500                  # vocab N-tile
NBT = (T * BL) // 128     # 5 bt-tiles of 128 (4 t x 32 b)
CH = 4                    # batches per score chunk
CHW = CH * P              # 784
KR = 8                    # resident W_ih enc k-chunks (of EC=16)

S_ENC = 2.0               # legacy awe output scale (undone via gates sigmoid/tanh scale)
S_FB = 32.0               # f_beta fp8 scale (undone in sigmoid scale)
S_EA = 32.0               # enc e4m3 scale for the DoubleRow awe matmuls
S_AL = 224.0              # alpha e4m3 scale (softmax in [0,1]; 224 <= e4m3 max 240)
# awe psum carries S_EA*S_AL; drain rescales to S_ENC-equivalent
S_AWE_FIX = S_ENC / (S_EA * S_AL)
S_WE = 1024.0             # W_enc_att e4m3 scale for P1
S_P1 = S_EA * S_WE        # P1 psum scale (undone in the att1 copy-out)

_BUILT = {}
DEBUG_DUMP = False


def _build(T_run, np_list):
    nc = bacc.Bacc("TRN2", target_bir_lowering=False, debug=False,
                   num_devices=NCORES)

    encP18_in = nc.dram_tensor("encP18", [128, 2, 8, BP], FP8E4, kind="ExternalInput").ap()
    encA8_in = nc.dram_tensor("encA8", [128, BL, 2, EC, 128], FP8E4, kind="ExternalInput").ap()
    wencT8_in = nc.dram_tensor("wencT8", [128, 2, 8, ATT], FP8E4, kind="ExternalInput").ap()
    bencp_in = nc.dram_tensor("bencp", [128, AT], F32, kind="ExternalInput").ap()
    wdecT_in = nc.dram_tensor("wdecT", [128, 5, ATT], BF16, kind="ExternalInput").ap()
    wvec_in = nc.dram_tensor("wvec", [128, AT], BF16, kind="ExternalInput").ap()
    wfbT8_in = nc.dram_tensor("wfbT8", [128, EC, 5, 128], FP8E3, kind="ExternalInput").ap()
    whhT_in = nc.dram_tensor("whhT", [128, 4, G], BF16, kind="ExternalInput").ap()
    wiheS_in = nc.dram_tensor("wiheS", [EC, 128, G], BF16, kind="ExternalInput").ap()
    wihembT_in = nc.dram_tensor("wihembT", [128, 5, G], BF16, kind="ExternalInput").ap()
    embT_in = nc.dram_tensor("embT", [128, 5, T * BL], BF16, kind="ExternalInput").ap()
    winitT_in = nc.dram_tensor("winitT", [128, 17, 2, DEC], BF16, kind="ExternalInput").ap()
    meanT_in = nc.dram_tensor("meanT", [128, 17, BL], BF16, kind="ExternalInput").ap()
    wfcT_in = nc.dram_tensor("wfcT", [DC, 128, V], BF16, kind="ExternalInput").ap()
    identf_in = nc.dram_tensor("identf", [128, 128], F32, kind="ExternalInput").ap()
    identb_in = nc.dram_tensor("identb", [128, 128], BF16, kind="ExternalInput").ap()

    pred_out = nc.dram_tensor("pred", [NBT, 128, V], F32, kind="ExternalOutput").ap()

    att1_dram = nc.dram_tensor("att1_scr", [AT, 128, BP], BF16).ap()
    epre_dram = nc.dram_tensor("epre_scr", [T, GT, 128, BL], BF16).ap()
    hist_dram = nc.dram_tensor("hist_scr", [T, DC, 128, BL], BF16).ap()

    NBT_run = (T_run * BL + 127) // 128

    with tile.TileContext(nc) as tc:
        with ExitStack() as octx:
            cpool = octx.enter_context(tc.tile_pool(name="const", bufs=1))

            wdecT_sb = cpool.tile([128, 5, ATT], BF16)
            nc.sync.dma_start(wdecT_sb[:], wdecT_in[:])
            wvec_sb = cpool.tile([128, AT], BF16)
            nc.sync.dma_start(wvec_sb[:], wvec_in[:])
            bencp_sb = cpool.tile([128, AT], F32)
            nc.sync.dma_start(bencp_sb[:], bencp_in[:])
            identf_sb = cpool.tile([128, 128], F32)
            nc.sync.dma_start(identf_sb[:], identf_in[:])
            identb_sb = cpool.tile([128, 128], BF16)
            nc.sync.dma_start(identb_sb[:], identb_in[:])

            h_sb = cpool.tile([128, 5, BL], BF16)       # 4 d-chunks + const [1;0]
            c_sb = cpool.tile([128, 4, BL], F32)
            # zero-padded diagonal alpha stationaries for DoubleRow awe:
            # col b of tile b holds alpha_b (hi / lo split); everything else 0.
            # pc1 rows >= P-128 stay 0 (enc there is zero-padded too).
            ahi_sb = cpool.tile([128, 2, BL, BL], FP8E4)
            alo_sb = cpool.tile([128, 2, BL, BL], FP8E4)
            nc.gpsimd.memset(ahi_sb[:], 0.0)
            nc.gpsimd.memset(alo_sb[:], 0.0)
            nc.gpsimd.memset(h_sb[:, 4, :], 0.0)
            nc.gpsimd.memset(h_sb[0:1, 4, :], 1.0)

            # ================= P1: att1 =================
            with ExitStack() as ctx:
                wp = ctx.enter_context(tc.tile_pool(name="p1w", bufs=1))
                pp = ctx.enter_context(tc.tile_pool(name="p1in", bufs=2))
                op_ = ctx.enter_context(tc.tile_pool(name="p1out", bufs=3))
                ps1 = ctx.enter_context(tc.tile_pool(name="p1ps", bufs=3, space=bass.MemorySpace.PSUM))

                wencT8_sb = wp.tile([128, 2, 8, ATT], FP8E4)
                nc.sync.dma_start(wencT8_sb[:], wencT8_in[:])
                NPAN = 16
                PAN = BP // NPAN  # 392
                for pan in range(NPAN):
                    et = pp.tile([128, 2, 8, PAN], FP8E4)
                    nc.sync.dma_start(
                        et[:], encP18_in[:, :, :, pan * PAN:(pan + 1) * PAN])
                    for at in range(AT):
                        ps = ps1.tile([128, PAN], F32)
                        for ep in range(8):
                            nc.tensor.matmul(
                                ps[:],
                                wencT8_sb[:, :, ep, at * 128:(at + 1) * 128],
                                et[:, :, ep, :],
                                start=(ep == 0), stop=(ep == 7), perf_mode=DR)
                        ob = op_.tile([128, PAN], BF16)
                        nc.vector.tensor_scalar(
                            ob[:], ps[:], 1.0 / S_P1, bencp_sb[:, at:at + 1],
                            OP.mult, OP.add)
                        nc.sync.dma_start(
                            att1_dram[at, :, pan * PAN:(pan + 1) * PAN], ob[:])

            # ================= P2: h0 / c0 =================
            with ExitStack() as ctx:
                wp = ctx.enter_context(tc.tile_pool(name="p2w", bufs=1))
                ps2 = ctx.enter_context(tc.tile_pool(name="p2ps", bufs=2, space=bass.MemorySpace.PSUM))
                winit_sb = wp.tile([128, 17, 2, DEC], BF16)
                nc.sync.dma_start(winit_sb[:], winitT_in[:])
                mean_sb = wp.tile([128, 17, BL], BF16)
                nc.sync.dma_start(mean_sb[:], meanT_in[:])
                for hc in range(2):
                    for dc in range(DC):
                        ps = ps2.tile([128, BL], F32)
                        for ek in range(17):
                            nc.tensor.matmul(
                                ps[:], winit_sb[:, ek, hc, dc * 128:(dc + 1) * 128],
                                mean_sb[:, ek, :],
                                start=(ek == 0), stop=(ek == 16))
                        if hc == 0:
                            nc.vector.tensor_copy(h_sb[:, dc, :], ps[:])
                        else:
                            nc.vector.tensor_copy(c_sb[:, dc, :], ps[:])

            # ================= P3: E_pre (x2 scale folded in wihembT) ==========
            with ExitStack() as ctx:
                wp = ctx.enter_context(tc.tile_pool(name="p3w", bufs=1))
                op_ = ctx.enter_context(tc.tile_pool(name="p3o", bufs=3))
                ps3 = ctx.enter_context(tc.tile_pool(name="p3ps", bufs=3, space=bass.MemorySpace.PSUM))
                wihe_sb = wp.tile([128, 5, G], BF16)
                nc.sync.dma_start(wihe_sb[:], wihembT_in[:])
                embT_sb = wp.tile([128, 5, T * BL], BF16)
                nc.sync.dma_start(embT_sb[:], embT_in[:])
                HT = (T * BL) // 2  # 320
                for gt in range(GT):
                    for ns in range(2):
                        ps = ps3.tile([128, HT], F32)
                        for ek in range(5):
                            nc.tensor.matmul(
                                ps[:], wihe_sb[:, ek, gt * 128:(gt + 1) * 128],
                                embT_sb[:, ek, ns * HT:(ns + 1) * HT],
                                start=(ek == 0), stop=(ek == 4))
                        ob = op_.tile([128, HT], BF16)
                        nc.vector.tensor_copy(ob[:], ps[:])
                        nc.sync.dma_start(
                            epre_dram[ns * (T // 2):(ns + 1) * (T // 2), gt]
                            .rearrange("t g b -> g t b"),
                            ob[:].rearrange("g (t b) -> g t b", b=BL))

            # ================= recurrent loop =================
            with ExitStack() as ctx:
                rp = ctx.enter_context(tc.tile_pool(name="res", bufs=1))
                a1p = ctx.enter_context(tc.tile_pool(name="a1", bufs=2))
                wkp = ctx.enter_context(tc.tile_pool(name="wk", bufs=2))
                smp = ctx.enter_context(tc.tile_pool(name="sm", bufs=2))
                ptp = ctx.enter_context(tc.tile_pool(name="pt", bufs=1))
                sfp = ctx.enter_context(tc.tile_pool(name="sf", bufs=1))
                pp_att2 = ctx.enter_context(tc.tile_pool(name="psA", bufs=1, space=bass.MemorySpace.PSUM))
                pp_dot = ctx.enter_context(tc.tile_pool(name="psD", bufs=1, space=bass.MemorySpace.PSUM))
                pp_tp = ctx.enter_context(tc.tile_pool(name="psT", bufs=1, space=bass.MemorySpace.PSUM))
                pp_awe = ctx.enter_context(tc.tile_pool(name="psW", bufs=1, space=bass.MemorySpace.PSUM))
                pp_gb = ctx.enter_context(tc.tile_pool(name="psG", bufs=1, space=bass.MemorySpace.PSUM))
                pp_gs = ctx.enter_context(tc.tile_pool(name="psS", bufs=1, space=bass.MemorySpace.PSUM))
                EW = 512           # awe e-wave width (4 waves of 4 ec)
                NEG = EW // 512    # psum-bank-sized output groups per wave

                # resident payloads
                encA_sb = rp.tile([128, BL, 2, EC, 128], FP8E4)
                nc.sync.dma_start(encA_sb[:], encA8_in[:])
                wfb_sb = rp.tile([128, EC, 5, 128], FP8E3)
                nc.sync.dma_start(wfb_sb[:], wfbT8_in[:])
                whhT_sb = rp.tile([128, 4, G], BF16)
                nc.sync.dma_start(whhT_sb[:], whhT_in[:])

                for t in range(T_run):
                    NP_ = np_list[t]
                    NC_ = NP_ // CH

                    ep_t = smp.tile([128, GT, BL], BF16, tag="ep")
                    nc.sync.dma_start(
                        ep_t[:, :, 0:NP_],
                        epre_dram[t][:, :, 0:NP_].rearrange("gt g b -> g gt b"))

                    # --- att2 = W_dec @ h + b_dec  (raw, [a, b] layout)
                    att2_ps = pp_att2.tile([128, AT, BL], F32)
                    for at in range(AT):
                        for dk in range(5):
                            nc.tensor.matmul(
                                att2_ps[:, at, 0:NP_],
                                wdecT_sb[:, dk, at * 128:(at + 1) * 128],
                                h_sb[:, dk, 0:NP_],
                                start=(dk == 0), stop=(dk == 4))
                    att2_sb = smp.tile([128, AT, BL], BF16, tag="att2")
                    nc.vector.tensor_copy(
                        att2_sb[:, :, 0:NP_], att2_ps[:, :, 0:NP_])

                    # --- scores: z = relu(att1 + att2), s = w . z
                    # staged 2 chunks at a time, DMA-reshaped to [b, p] rows
                    scores_bp = smp.tile([BL, P], BF16, tag="scores")
                    scores_fl = sfp.tile([1, 2 * CHW], BF16, tag="scoresf")
                    NSUB = [512, 272]  # 784 split (psum bank limit)
                    for c in range(NC_):
                        a1 = a1p.tile([128, AT, CHW], BF16)
                        nc.sync.dma_start(
                            a1[:], att1_dram[:, :, c * CHW:(c + 1) * CHW]
                            .rearrange("a p j -> p a j"))
                        for at in range(AT):
                            nc.vector.tensor_tensor(
                                a1[:, at, :].rearrange("p (b q) -> p b q", q=P),
                                a1[:, at, :].rearrange("p (b q) -> p b q", q=P),
                                att2_sb[:, at, c * CH:(c + 1) * CH]
                                .rearrange("p (b u) -> p b u", u=1)
                                .broadcast_to([128, CH, P]),
                                OP.add)
                        nc.scalar.activation(
                            a1[:].rearrange("p a j -> p (a j)"),
                            a1[:].rearrange("p a j -> p (a j)"), AF.Relu)
                        off = 0
                        so = (c % 2) * CHW
                        for si, nsub in enumerate(NSUB):
                            ps = pp_dot.tile([1, 512], F32)
                            for at in range(AT):
                                nc.tensor.matmul(
                                    ps[:, 0:nsub], wvec_sb[:, at:at + 1],
                                    a1[:, at, off:off + nsub],
                                    start=(at == 0), stop=(at == AT - 1))
                            if si % 2 == 0:
                                nc.scalar.activation(
                                    scores_fl[:, so + off:so + off + nsub],
                                    ps[:, 0:nsub], AF.Copy)
                            else:
                                nc.vector.tensor_copy(
                                    scores_fl[:, so + off:so + off + nsub],
                                    ps[:, 0:nsub])
                            off += nsub
                        if c % 2 == 1:
                            nc.sync.dma_start(
                                scores_bp[(c - 1) * CH:(c + 1) * CH, :],
                                scores_fl[:, 0:2 * CHW]
                                .rearrange("u (b q) -> u b q", q=P))
                        elif c == NC_ - 1:
                            nc.sync.dma_start(
                                scores_bp[c * CH:(c + 1) * CH, :],
                                scores_fl[:, 0:CHW]
                                .rearrange("u (b q) -> u b q", q=P))

                    # --- softmax over p
                    mx = smp.tile([BL, 1], F32, tag="mx")
                    nc.vector.tensor_reduce(mx[0:NP_, :], scores_bp[0:NP_, :],
                                            AX.X, OP.max, negate=True)
                    ex = smp.tile([BL, P], F32, tag="ex")
                    sume = smp.tile([BL, 1], F32, tag="sume")
                    nc.scalar.activation(ex[0:NP_, :], scores_bp[0:NP_, :], AF.Exp,
                                         bias=mx[0:NP_, :], accum_out=sume[0:NP_, :])
                    rc = smp.tile([BL, 1], F32, tag="rc")
                    nc.vector.reciprocal(rc[0:NP_, :], sume[0:NP_, :])
                    al = ex
                    nc.vector.tensor_scalar(al[0:NP_, :], ex[0:NP_, :],
                                            rc[0:NP_, :], None, OP.mult)

                    # --- transpose alpha -> [p, b], scale, split hi/lo into the
                    # diagonal e4m3 stationaries (padded rows stay 0)
                    tp = pp_tp.tile([128, 2 * BL], F32)
                    nc.tensor.transpose(tp[0:128, 0:NP_], al[0:NP_, 0:128],
                                        identf_sb[0:NP_, 0:NP_])
                    nc.tensor.transpose(tp[0:P - 128, BL:BL + NP_], al[0:NP_, 128:P],
                                        identf_sb[0:NP_, 0:NP_])
                    t1 = smp.tile([128, 2, BL], BF16, tag="t1")
                    nc.vector.tensor_scalar(t1[:, 0, 0:NP_], tp[:, 0:NP_],
                                            S_AL, None, OP.mult)
                    nc.vector.tensor_scalar(t1[0:P - 128, 1, 0:NP_],
                                            tp[0:P - 128, BL:BL + NP_],
                                            S_AL, None, OP.mult)
                    dhi = ahi_sb[:].rearrange("k two b c -> k two (b c)")[:, :, ::BL + 1]
                    dlo = alo_sb[:].rearrange("k two b c -> k two (b c)")[:, :, ::BL + 1]
                    nc.vector.tensor_copy(dhi[:, 0, 0:NP_], t1[:, 0, 0:NP_])
                    nc.vector.tensor_copy(dhi[0:P - 128, 1, 0:NP_],
                                          t1[0:P - 128, 1, 0:NP_])
                    nc.vector.tensor_tensor(t1[:, 0, 0:NP_], t1[:, 0, 0:NP_],
                                            dhi[:, 0, 0:NP_], OP.subtract)
                    nc.vector.tensor_tensor(t1[0:P - 128, 1, 0:NP_],
                                            t1[0:P - 128, 1, 0:NP_],
                                            dhi[0:P - 128, 1, 0:NP_], OP.subtract)
                    nc.vector.tensor_copy(dlo[:, 0, 0:NP_], t1[:, 0, 0:NP_])
                    nc.vector.tensor_copy(dlo[0:P - 128, 1, 0:NP_],
                                          t1[0:P - 128, 1, 0:NP_])

                    # --- awe via DoubleRow: alpha-diag stationary, enc moving.
                    # psum [BL, 512] groups accumulate row b = awe_b; per wave:
                    # drain -> transpose back to [128, ec, b].
                    aweT = ptp.tile([128, EC, BL], BF16, tag="aweT")
                    ECW = EW // 128    # ec chunks per eg-chain (4)
                    NCH = ENC // EW    # chains (4)
                    aw_ps0 = pp_awe.tile([BL, EW], F32, tag="aw0")
                    aw_ps1 = pp_awe.tile([BL, EW], F32, tag="aw1")
                    aw_sb0 = smp.tile([BL, EW], BF16, tag="ab0")
                    aw_sb1 = smp.tile([BL, EW], BF16, tag="ab1")
                    aps = [aw_ps0, aw_ps1]
                    abuf = [aw_sb0, aw_sb1]
                    tb = pp_tp.tile([128, ECW, BL], BF16, tag="tb")

                    def emit_transpose(eg):
                        sb = abuf[eg % 2]
                        for blk in range(ECW):
                            nc.tensor.transpose(
                                tb[:, blk, :], sb[:, blk * 128:(blk + 1) * 128],
                                identb_sb[0:BL, 0:BL])
                        nc.vector.tensor_copy(
                            aweT[:, eg * ECW:(eg + 1) * ECW, :], tb[:])

                    for eg in range(NCH):
                        ps = aps[eg % 2]
                        ecs = eg * ECW
                        for b in range(NP_):
                            nc.tensor.matmul(
                                ps[:], ahi_sb[:, :, b, :],
                                encA_sb[:, b, :, ecs:ecs + ECW, :],
                                start=(b == 0), stop=False, perf_mode=DR)
                            nc.tensor.matmul(
                                ps[:], alo_sb[:, :, b, :],
                                encA_sb[:, b, :, ecs:ecs + ECW, :],
                                start=False, stop=(b == NP_ - 1), perf_mode=DR)
                        # drain chain (rescale to S_ENC-equivalent), DVE/ACT alternating
                        if eg % 2 == 0:
                            nc.vector.tensor_scalar(
                                abuf[0][:], ps[:], S_AWE_FIX, None, OP.mult)
                        else:
                            nc.scalar.activation(
                                abuf[1][:], ps[:], AF.Copy, scale=S_AWE_FIX)
                        if eg >= 1:
                            emit_transpose(eg - 1)
                    emit_transpose(NCH - 1)

                    # --- f_beta gate (resident fp8, x S_FB; undone in sigmoid)
                    gb_ps = pp_gb.tile([128, EC, BL], F32, tag="gb")
                    for et in range(EC):
                        for dk in range(5):
                            nc.tensor.matmul(
                                gb_ps[:, et, 0:NP_],
                                wfb_sb[:, et, dk, :], h_sb[:, dk, 0:NP_],
                                start=(dk == 0), stop=(dk == 4))
                    gate_s = ptp.tile([128, EC, BL], BF16, tag="gate")
                    nc.scalar.activation(
                        gate_s[:, :, 0:NP_], gb_ps[:, :, 0:NP_],
                        AF.Sigmoid, scale=1.0 / S_FB)
                    xenc = ptp.tile([128, EC, BL], BF16, tag="xenc")
                    nc.vector.tensor_tensor(
                        xenc[:, :, 0:NP_], gate_s[:, :, 0:NP_],
                        aweT[:, :, 0:NP_], OP.mult)

                    # --- gates = W_ihenc @ xenc + W_hh @ h + E_pre (all x2)
                    g_ps = pp_gs.tile([128, GT, BL], F32, tag="gps")
                    for k in range(EC):
                        wk = wkp.tile([128, G], BF16)
                        nc.sync.dma_start(wk[:], wiheS_in[k])
                        for gt in range(GT):
                            nc.tensor.matmul(
                                g_ps[:, gt, 0:NP_],
                                wk[:, gt * 128:(gt + 1) * 128],
                                xenc[:, k, 0:NP_],
                                start=(k == 0 and gt == 0), stop=False)
                    for dk in range(4):
                        for gt in range(GT):
                            nc.tensor.matmul(
                                g_ps[:, gt, 0:NP_],
                                whhT_sb[:, dk, gt * 128:(gt + 1) * 128],
                                h_sb[:, dk, 0:NP_],
                                start=False, stop=False)
                    for gt in range(GT):
                        nc.tensor.matmul(
                            g_ps[:, gt, 0:NP_],
                            identb_sb[:], ep_t[:, gt, 0:NP_],
                            start=False, stop=(gt == GT - 1))

                    # --- pointwise LSTM (scale 0.5 undoes the x2)
                    sig_if = ptp.tile([128, 8, BL], F32, tag="sif")
                    nc.scalar.activation(
                        sig_if[:, :, 0:NP_], g_ps[:, 0:8, 0:NP_],
                        AF.Sigmoid, scale=0.5)
                    tanh_g = ptp.tile([128, 4, BL], F32, tag="tg")
                    nc.scalar.activation(
                        tanh_g[:, :, 0:NP_], g_ps[:, 8:12, 0:NP_],
                        AF.Tanh, scale=0.5)
                    sig_o = ptp.tile([128, 4, BL], F32, tag="so")
                    nc.scalar.activation(
                        sig_o[:, :, 0:NP_], g_ps[:, 12:16, 0:NP_],
                        AF.Sigmoid, scale=0.5)

                    fc_ = ptp.tile([128, 4, BL], F32, tag="fc")
                    nc.vector.tensor_tensor(
                        fc_[:, :, 0:NP_], sig_if[:, 4:8, 0:NP_],
                        c_sb[:, :, 0:NP_], OP.mult)
                    ig_ = ptp.tile([128, 4, BL], F32, tag="ig")
                    nc.vector.tensor_tensor(
                        ig_[:, :, 0:NP_], sig_if[:, 0:4, 0:NP_],
                        tanh_g[:, :, 0:NP_], OP.mult)
                    nc.vector.tensor_tensor(
                        c_sb[:, :, 0:NP_], fc_[:, :, 0:NP_],
                        ig_[:, :, 0:NP_], OP.add)
                    tanh_c = ptp.tile([128, 4, BL], F32, tag="tc")
                    nc.scalar.activation(
                        tanh_c[:, :, 0:NP_], c_sb[:, :, 0:NP_], AF.Tanh)
                    h_new = ptp.tile([128, 4, BL], BF16, tag="hn")
                    nc.vector.tensor_tensor(
                        h_new[:, :, 0:NP_], sig_o[:, :, 0:NP_],
                        tanh_c[:, :, 0:NP_], OP.mult)
                    nc.vector.tensor_copy(h_sb[:, 0:4, 0:NP_], h_new[:, :, 0:NP_])
                    nc.sync.dma_start(
                        hist_dram[t][:, :, 0:NP_].rearrange("k d b -> d k b"),
                        h_new[:, :, 0:NP_])

                if DEBUG_DUMP:
                    dbg_scores = nc.dram_tensor("dbg_scores", [BL, P], BF16, kind="ExternalOutput").ap()
                    dbg_ahi = nc.dram_tensor("dbg_ahi", [128, 2, BL, BL], FP8E4, kind="ExternalOutput").ap()
                    dbg_alo = nc.dram_tensor("dbg_alo", [128, 2, BL, BL], FP8E4, kind="ExternalOutput").ap()
                    dbg_aweT = nc.dram_tensor("dbg_aweT", [128, EC, BL], BF16, kind="ExternalOutput").ap()
                    dbg_h = nc.dram_tensor("dbg_h", [128, 5, BL], BF16, kind="ExternalOutput").ap()
                    dbg_xenc = nc.dram_tensor("dbg_xenc", [128, EC, BL], BF16, kind="ExternalOutput").ap()
                    nc.sync.dma_start(dbg_scores[:], scores_bp[:])
                    nc.sync.dma_start(dbg_ahi[:], ahi_sb[:])
                    nc.sync.dma_start(dbg_alo[:], alo_sb[:])
                    nc.sync.dma_start(dbg_aweT[:], aweT[:])
                    nc.sync.dma_start(dbg_h[:], h_sb[:])
                    nc.sync.dma_start(dbg_xenc[:], xenc[:])

            # ================= P5: vocab projection =================
            with ExitStack() as ctx:
                hp = ctx.enter_context(tc.tile_pool(name="p5h", bufs=1))
                wp = ctx.enter_context(tc.tile_pool(name="p5w", bufs=2))
                op_ = ctx.enter_context(tc.tile_pool(name="p5o", bufs=3))
                ps5 = ctx.enter_context(tc.tile_pool(name="p5ps", bufs=4, space=bass.MemorySpace.PSUM))
                hh_sb = hp.tile([128, DC, T, BL], BF16)
                for dc in range(DC):
                    nc.sync.dma_start(
                        hh_sb[:, dc, 0:T_run, :],
                        hist_dram[0:T_run, dc].rearrange("t d b -> d t b"))
                for vt in range(V // VT):
                    wv = wp.tile([128, DC, VT], BF16)
                    nc.sync.dma_start(
                        wv[:], wfcT_in[:, :, vt * VT:(vt + 1) * VT]
                        .rearrange("k d v -> d k v"))
                    for btc in range(NBT_run):
                        ps = ps5.tile([128, VT], F32)
                        for dc in range(DC):
                            nc.tensor.matmul(
                                ps[:],
                                hh_sb[:, dc, btc * 4:(btc + 1) * 4, :]
                                .rearrange("d t b -> d (t b)"),
                                wv[:, dc, :],
                                start=(dc == 0), stop=(dc == DC - 1))
                        ob = op_.tile([128, VT], F32)
                        nc.vector.tensor_copy(ob[:], ps[:])
                        nc.sync.dma_start(
                            pred_out[btc, :, vt * VT:(vt + 1) * VT], ob[:])

    nc.compile()
    return nc


def _get_nc(T_run, np_list):
    key = (T_run, tuple(np_list))
    if key not in _BUILT:
        _BUILT[key] = _build(T_run, np_list)
    return _BUILT[key]


def _dither_fp8(x, s, npdt, maxv):
    """Quantize x*s to fp8 so that per-(b,e) pixel-column error sums ~0."""
    x = np.asarray(x, np.float32)
    xq8 = np.clip(x * s, -maxv, maxv).astype(npdt)
    xf = xq8.astype(np.float32)
    delta = xf - x * s
    colsum = delta.sum(axis=1)
    sgn = np.sign(colsum)[:, None, :]
    i = xq8.view(np.uint8).astype(np.int16)
    up = (xf >= 0) == (-sgn > 0)
    i2 = np.clip(np.where(up, i + 1, i - 1), 0, 255).astype(np.uint8)
    cand = i2.view(npdt).astype(np.float32)
    cand = np.where(np.isfinite(cand), cand, xf)
    cost = cand - xf
    elig = (np.sign(cost) == -sgn) & (np.abs(cost) > 0)
    ch = np.where(elig, cost, 0.0)
    cum = np.cumsum(ch, axis=1)
    mask = (np.abs(cum) <= np.abs(colsum)[:, None, :]) & elig
    return np.where(mask, cand, xf).astype(npdt)


def _dither_e3m4(x, s):
    return _dither_fp8(x, s, NP_FP8E3, 15.5)


def _dither_e4m3(x, s):
    return _dither_fp8(x, s, NP_FP8E4, 224.0)


def _prep(inputs):
    enc = np.asarray(inputs["encoder_out"], np.float32)
    caps = np.asarray(inputs["encoded_captions"]).astype(np.int64)
    lens = np.asarray(inputs["caption_lengths"]).astype(np.int64)[:, 0]
    emb = np.asarray(inputs["emb"], np.float32)
    W_enc_att = np.asarray(inputs["W_enc_att"], np.float32)
    b_enc_att = np.asarray(inputs["b_enc_att"], np.float32)
    W_dec_att = np.asarray(inputs["W_dec_att"], np.float32)
    b_dec_att = np.asarray(inputs["b_dec_att"], np.float32)
    W_full_att = np.asarray(inputs["W_full_att"], np.float32)
    b_full_att = np.asarray(inputs["b_full_att"], np.float32)
    W_init_h = np.asarray(inputs["W_init_h"], np.float32)
    b_init_h = np.asarray(inputs["b_init_h"], np.float32)
    W_init_c = np.asarray(inputs["W_init_c"], np.float32)
    b_init_c = np.asarray(inputs["b_init_c"], np.float32)
    W_f_beta = np.asarray(inputs["W_f_beta"], np.float32)
    b_f_beta = np.asarray(inputs["b_f_beta"], np.float32)
    W_fc = np.asarray(inputs["W_fc"], np.float32)
    b_fc = np.asarray(inputs["b_fc"], np.float32)
    W_ih = np.asarray(inputs["W_ih"], np.float32)
    W_hh = np.asarray(inputs["W_hh"], np.float32)
    b_ih = np.asarray(inputs["b_ih"], np.float32)
    b_hh = np.asarray(inputs["b_hh"], np.float32)

    sort_ind = np.argsort(-lens, kind="stable")
    dec_len_s = (lens[sort_ind] - 1).astype(np.int64)     # desc
    T_run = int(dec_len_s.max())
    Ng = np.array([(dec_len_s > t).sum() for t in range(T_run)])
    np_list = [min(BL, int(-(-int(n) // NCORES) + 3) // 4 * 4) for n in Ng]

    # round-robin core assignment over the sorted order
    core_ranks = [sort_ind[c::NCORES] for c in range(NCORES)]   # global batch ids
    core_dlen = [dec_len_s[c::NCORES] for c in range(NCORES)]

    mean_enc = enc.mean(axis=1)                                  # [B, ENC]

    def bf(x):
        return np.ascontiguousarray(x).astype(NP_BF16)

    # ---- shared weight payloads
    we8 = _dither_fp8(W_enc_att.T[None], S_WE, NP_FP8E4, 224.0)[0]  # [ENC,ATT]
    wencT8 = np.ascontiguousarray(
        we8.reshape(8, 2, 128, ATT).transpose(2, 1, 0, 3))
    bencp = np.ascontiguousarray(b_enc_att.reshape(AT, 128).T).astype(np.float32)
    wvec = bf(W_full_att[0].reshape(AT, 128).T)

    def kext(WT, bias, kchunks):
        Wc = WT.reshape(kchunks, 128, WT.shape[1])
        ext = np.zeros((1, 128, WT.shape[1]), np.float32)
        ext[0, 0, :] = bias
        return np.concatenate([Wc, ext], axis=0)

    wdecT = bf(kext(W_dec_att.T, b_dec_att, 4).transpose(1, 0, 2))    # [128,5,ATT]
    wfb_scaled = np.clip(kext(W_f_beta.T * S_FB, b_f_beta * S_FB, 4),
                         -15.5, 15.5)                                 # [5,128,ENC]
    wfbT8 = np.ascontiguousarray(
        wfb_scaled.reshape(5, 128, EC, 128).transpose(1, 2, 0, 3)).astype(NP_FP8E3)
    whhT = bf((2.0 * W_hh).T.reshape(4, 128, G).transpose(1, 0, 2))   # [128,4,G]
    wiheS = bf(W_ih[:, EMB:].T.reshape(EC, 128, G))                   # [EC,128,G]
    wihembT = bf(kext(2.0 * W_ih[:, :EMB].T, 2.0 * (b_ih + b_hh), 4)
                 .transpose(1, 0, 2))                                 # [128,5,G]
    winit = np.stack([W_init_h.T, W_init_c.T], axis=1)                # [ENC,2,DEC]
    winitc = winit.reshape(EC, 128, 2, DEC)
    wext = np.zeros((1, 128, 2, DEC), np.float32)
    wext[0, 0, 0, :] = b_init_h
    wext[0, 0, 1, :] = b_init_c
    winitT = bf(np.concatenate([winitc, wext], axis=0).transpose(1, 0, 2, 3))
    wfcT = bf(W_fc.T.reshape(DC, 128, V))
    identf = np.eye(128, dtype=np.float32)
    identb = np.eye(128, dtype=np.float32).astype(NP_BF16)

    in_maps = []
    for cidx in range(NCORES):
        gids = core_ranks[cidx]
        e = enc[gids]                                             # [BL,P,ENC]
        e8p = _dither_fp8(e.reshape(BL * P, ENC, 1), S_EA, NP_FP8E4, 224.0)
        encP18 = np.ascontiguousarray(
            e8p.reshape(BL * P, ENC).T.reshape(8, 2, 128, BP)
            .transpose(2, 1, 0, 3))
        ep = np.zeros((BL, 256, ENC), np.float32)
        ep[:, :P, :] = e
        e8 = _dither_e4m3(ep, S_EA)                               # [BL,256,ENC] e4m3
        encA8 = np.ascontiguousarray(
            e8.reshape(BL, 2, 128, EC, 128).transpose(2, 0, 1, 3, 4))
        emb_seq = emb[caps[gids][:, :T]]                          # [BL,T,EMB]
        es = emb_seq.transpose(2, 1, 0).reshape(4, 128, T * BL)
        esx = np.concatenate([es, np.zeros((1, 128, T * BL), np.float32)], axis=0)
        esx[4, 0, :] = 1.0
        embT = bf(esx.transpose(1, 0, 2))                         # [128,5,T*BL]
        mn = mean_enc[gids].T.reshape(EC, 128, BL)
        mnx = np.concatenate([mn, np.zeros((1, 128, BL), np.float32)], axis=0)
        mnx[16, 0, :] = 1.0
        meanT = bf(mnx.transpose(1, 0, 2))                        # [128,17,BL]
        in_maps.append(dict(
            encP18=encP18, encA8=encA8, wencT8=wencT8, bencp=bencp, wdecT=wdecT,
            wvec=wvec, wfbT8=wfbT8, whhT=whhT, wiheS=wiheS,
            wihembT=wihembT, embT=embT, winitT=winitT, meanT=meanT,
            wfcT=wfcT, identf=identf, identb=identb))
    return in_maps, (T_run, np_list, core_ranks, core_dlen), b_fc


def kernel(**inputs):
    in_maps, meta, b_fc = _prep(inputs)
    T_run, np_list, core_ranks, core_dlen = meta
    nc = _get_nc(T_run, np_list)
    res = run_bass_kernel_spmd(nc, in_maps, core_ids=list(range(NCORES)))
    out = np.zeros((B, T, V), np.float32)
    for cidx in range(NCORES):
        p = res.results[cidx]["pred"]                  # [NBT,128,V]
        p = p.reshape(NBT * 128, V)[:T * BL].reshape(T, BL, V)
        dl = core_dlen[cidx]
        for i in range(BL):
            rank = cidx + NCORES * i
            n = int(dl[i])
            out[rank, :n] = p[:n, i]
    if np.any(b_fc):
        # masked bias add (only active steps)
        act = np.zeros((B, T, 1), np.float32)
        for cidx in range(NCORES):
            for i in range(BL):
                act[cidx + NCORES * i, :int(core_dlen[cidx][i])] = 1.0
        out += act * b_fc[None, None, :]
    return np.ascontiguousarray(out, dtype=np.float32)

